# revision 15
# baseline (speedup 1.0000x reference)
"""CrossDomainClassSpecificFrequencyMixStyle on 8 Trainium2 NeuronCores.

Contract: kernel(**inputs) takes FULL unsharded inputs (as produced by
reference.setup_inputs) and returns the FULL [B, N, C] float32 output.

Math (per sample b, channel c):
    mu[b,c], sig[b,c] = stats of x[b, :, c] over N   (unbiased var + eps, sqrt)
    idx[b] = partner sample (same class, different domain, max noise) else b
    a[b] = alpha_u[b] * 0.5
    mu_mix  = a*mu + (1-a)*mu[idx]
    sig_mix = a*sig + (1-a)*sig[idx]
    out = (x - mu)/sig * sig_mix + mu_mix  =  x * scale + bias
        scale = sig_mix/sig ;  bias = mu_mix - mu*scale

Distribution: data-parallel over B (8 samples per core). Partner selection is
computed on host and shipped as a one-hot matrix; per-sample (mu, sig) are
AllGathered across the 8 cores.

v3 design ("paired" channel-major layout):
  - Host ships x as fp16 in [S/2, 128, N] pair-tiles: partition (a*64+c) of
    pair-tile t holds channel c of sample 2t+a. Output comes back the same
    way and is un-transposed + upcast on host. Total device HBM traffic:
    16 MiB load + 16 MiB store per core.
  - All 4 pair-tiles stay RESIDENT in SBUF (128 KiB/partition).
  - Per-(sample, channel) stats are per-PARTITION free-dim reductions:
    DVE tensor_reduce / tensor_tensor_reduce on tiles 0-1, ScalarE
    activation accum_out (Copy for sum, Square for sumsq) on tiles 2-3.
  - Row-layout [S, 2C] stats for the collective are produced by a DMA whose
    DRAM-side access pattern does the (a,c,k,t) -> rows permutation; own +
    partner row stats come back from the gathered [B, 2C] via one-hot
    matmuls; tiny row math forms scale/bias rows; 4 small PE matmuls with
    host-shipped selectors turn rows into per-partition scale/bias columns.
  - Apply is ONE DVE tensor_scalar (mult, add) per pair-tile -- fp16
    packed+SBUF hits the 4x DVE mode; f32 per-partition scalars are allowed.
    Stores alternate the two HWDGE rings.
"""

import dataclasses
import sys

sys.path.insert(0, "/opt/trn_rl_repo")

import numpy as np

import concourse.bass as bass
import concourse.tile as tile
from concourse import bacc, mybir
from concourse.bass_utils import run_bass_kernel_spmd

F32 = mybir.dt.float32
F16 = mybir.dt.float16

R = 8          # cores
B = 64         # batch
C = 64         # channels
S = B // R     # samples per core (8)
T = S // 2     # pair-tiles per core (4)
EPS = 1e-6
ALPHA_MAX = 0.5
P = 128        # partitions

# risky-construct switches (bisected on hardware)
USE_ACT_STATS = False   # ScalarE activation accum_out for tiles 2-3
USE_OFFSET_MM = False   # PE matmul writing PSUM at partition offset 64


def build_nc(N=16384, CH=8192, n_cores=R, reps=1, loop_iters=0,
             variant="full", bench_sink=False):
    """Build + bacc-compile the SPMD program. N = tokens per sample,
    CH = stats chunk size (fp16 elems per partition per instruction).
    reps: how many full pipelines (with collective) to unroll.
    loop_iters: if >0, additionally emit a For_i loop running the pipeline
    body loop_iters times WITHOUT the collective (for slope timing).
    variant (loop body only): full | copy | stats | apply."""
    assert N % CH == 0
    NCH = N // CH              # stats chunks per pair-tile

    nc = bacc.Bacc("TRN2", target_bir_lowering=False, debug=False,
                   num_devices=n_cores)
    xs_d = nc.dram_tensor("xs", [T, P, N], F16, kind="ExternalInput")
    pt_d = nc.dram_tensor("pt", [B, S], F32, kind="ExternalInput")
    ow_d = nc.dram_tensor("ow", [B, S], F32, kind="ExternalInput")
    ra_d = nc.dram_tensor("ra", [S, 8], F32, kind="ExternalInput")
    ey_d = nc.dram_tensor("ey", [P, P], F32, kind="ExternalInput")
    al_d = nc.dram_tensor("al", [S, 1], F32, kind="ExternalInput")
    out_kind = "Internal" if bench_sink else "ExternalOutput"
    out_d = nc.dram_tensor("out", [T, P, N], F16, kind=out_kind)
    # tiny debug output: per-sample mu||sig (also a cheap D2H sync point)
    st_d = nc.dram_tensor("stats_out", [S, 2 * C], F32, kind="ExternalOutput")

    with tile.TileContext(nc) as tc:
        with (
            tc.tile_pool(name="res", bufs=1) as res_pool,
            tc.tile_pool(name="scr", bufs=3) as scr_pool,
            tc.tile_pool(name="small", bufs=1) as small,
            tc.tile_pool(name="pmisc", bufs=2, space="PSUM") as pmisc,
            tc.tile_pool(name="ppc", bufs=1, space="PSUM") as ppc,
            tc.tile_pool(name="dram", bufs=1, space="DRAM") as dram,
        ):
            # ---- loop-invariant constants ----
            pt_sb = small.tile([B, S], F32, tag="ptsb")
            nc.sync.dma_start(pt_sb[:], pt_d[:])
            ow_sb = small.tile([B, S], F32, tag="owsb")
            nc.sync.dma_start(ow_sb[:], ow_d[:])
            ra_sb = small.tile([S, 8], F32, tag="rasb")
            nc.sync.dma_start(ra_sb[:], ra_d[:])
            ey_sb = small.tile([P, P], F32, tag="eysb")
            nc.sync.dma_start(ey_sb[:], ey_d[:])
            al_sb = small.tile([S, 1], F32, tag="alsb")
            nc.sync.dma_start(al_sb[:], al_d[:])
            eps_c = small.tile([P, 1], F32, tag="epsc")
            nc.vector.memset(eps_c[:], EPS)
            res = [res_pool.tile([P, N], F16, tag=f"res{t}", name=f"res{t}")
                   for t in range(T)]
            sums = small.tile([P, T], F32, tag="sums")
            sqs = small.tile([P, 2 * T], F32, tag="sqs")  # chunk accums
            st_pair = small.tile([P, 2 * T], F32, tag="stpair")
            cc_in = dram.tile([S, 2 * C], F32, tag="ccin")
            cc_out = dram.tile([B, 2 * C], F32, tag="ccout")
            pc_d = dram.tile([64, 2 * T], F32, tag="pcd")

            def emit(do_collective, var="full"):
                if var == "copy":
                    for t in range(T):
                        nc.sync.dma_start(res[t][:], xs_d[t])
                    for t in range(T):
                        eng = nc.scalar if t % 2 else nc.sync
                        eng.dma_start(out_d[t], res[t][:])
                    st = small.tile([S, 2 * C], F32, tag="musig")
                    nc.vector.memset(st[:], 1.0)
                    nc.sync.dma_start(st_d[:], st[:])
                    return
                # ---------------- phase A: per-partition stats ------------
                if var != "apply":
                    for t in range(T):
                        # first load takes the otherwise-idle ACT ring so
                        # the last load lands ~25% earlier
                        leng = nc.scalar if t == 0 else nc.sync
                        leng.dma_start(res[t][:], xs_d[t])
                        if t < 3:
                            # DVE: per-partition sum (full-tile reduce)
                            nc.vector.tensor_reduce(
                                out=sums[:, t:t + 1], in_=res[t][:],
                                axis=mybir.AxisListType.X,
                                op=mybir.AluOpType.add)
                            # ScalarE: sumsq via Square+accum_out
                            for k in range(NCH):
                                scr = scr_pool.tile([P, CH], F16, tag="scr")
                                nc.scalar.activation(
                                    scr[:], res[t][:, bass.ts(k, CH)],
                                    mybir.ActivationFunctionType.Square,
                                    accum_out=sqs[:, NCH * t + k:
                                                  NCH * t + k + 1])
                        else:
                            # last tile rebalanced: sum on ScalarE
                            # (Copy+accum), sumsq on DVE (square+reduce)
                            ac3 = small.tile([P, 2], F32, tag="ac3")
                            for k in range(NCH):
                                scr = scr_pool.tile([P, CH], F16, tag="scr")
                                nc.scalar.activation(
                                    scr[:], res[t][:, bass.ts(k, CH)],
                                    mybir.ActivationFunctionType.Copy,
                                    accum_out=ac3[:, k:k + 1])
                                scr2 = scr_pool.tile([P, CH], F16,
                                                     tag="scr")
                                nc.vector.tensor_tensor(
                                    out=scr2[:],
                                    in0=res[t][:, bass.ts(k, CH)],
                                    in1=res[t][:, bass.ts(k, CH)],
                                    op=mybir.AluOpType.mult)
                                nc.vector.tensor_reduce(
                                    out=sqs[:, NCH * t + k:
                                            NCH * t + k + 1],
                                    in_=scr2[:],
                                    axis=mybir.AxisListType.X,
                                    op=mybir.AluOpType.add)
                            nc.vector.tensor_tensor(
                                out=sums[:, t:t + 1], in0=ac3[:, 0:1],
                                in1=ac3[:, 1:2], op=mybir.AluOpType.add)
                    # fold the NCH sumsq partials per tile: sq4[:, t]
                    sq4 = small.tile([P, T], F32, tag="sq4")
                    if NCH == 2:
                        e0 = dataclasses.replace(
                            sqs[:], ap=[sqs[:].ap[0], [2, T]])
                        e1 = dataclasses.replace(
                            sqs[:], ap=[sqs[:].ap[0], [2, T]],
                            offset=sqs[:].offset + 1)
                        nc.vector.tensor_tensor(out=sq4[:], in0=e0, in1=e1,
                                                op=mybir.AluOpType.add)
                    else:
                        assert NCH == 1
                        nc.vector.tensor_copy(sq4[:], sqs[:, 0:T])

                    # st_pair cols: t -> mu_t ; T+t -> sig_t
                    mu4 = st_pair[:, 0:T]
                    nc.vector.tensor_scalar(out=mu4, in0=sums[:],
                                            scalar1=1.0 / N, scalar2=None,
                                            op0=mybir.AluOpType.mult)
                    q4 = small.tile([P, T], F32, tag="q4")
                    nc.vector.tensor_tensor(out=q4[:], in0=mu4, in1=mu4,
                                            op=mybir.AluOpType.mult)
                    nc.vector.tensor_scalar(out=q4[:], in0=q4[:],
                                            scalar1=-float(N), scalar2=None,
                                            op0=mybir.AluOpType.mult)
                    nc.vector.tensor_tensor(out=q4[:], in0=q4[:],
                                            in1=sq4[:],
                                            op=mybir.AluOpType.add)
                    # sig = sqrt(q/(N-1) + eps)
                    nc.scalar.activation(st_pair[:, T:2 * T], q4[:],
                                         mybir.ActivationFunctionType.Sqrt,
                                         bias=eps_c[:], scale=1.0 / (N - 1))

                    # paired [a*64+c, k*T+t] -> rows cc_in[2t+a, k*64+c]:
                    # PE-transpose st_pair, then 4 ordinary strided DMAs
                    tt_ps = pmisc.tile([2 * T, P], F32, tag="ttps")
                    nc.tensor.matmul(tt_ps[:], st_pair[:], ey_sb[:],
                                     start=True, stop=True)
                    tts = small.tile([2 * T, P], F32, tag="tts")
                    nc.vector.tensor_copy(tts[:], tt_ps[:])
                    for a in range(2):
                        for k in range(2):
                            src = tts[k * T:(k + 1) * T,
                                      a * 64:(a + 1) * 64]
                            dst = cc_in[:]
                            dst = dataclasses.replace(
                                dst, ap=[[2 * 2 * C, T], [1, 64]],
                                offset=dst.offset + a * 2 * C + k * C)
                            nc.sync.dma_start(dst, src)

                if do_collective:
                    nc.gpsimd.collective_compute(
                        "AllGather", mybir.AluOpType.bypass,
                        replica_groups=[list(range(n_cores))],
                        ins=[cc_in.opt()], outs=[cc_out.opt()],
                    )
                gath = small.tile([B, 2 * C], F32, tag="gath")
                nc.sync.dma_start(gath[:], cc_out[:])

                # -------------- phase B: rows -> scale/bias ---------------
                # own + partner row stats via one-hot matmuls
                ps_own = pmisc.tile([S, 2 * C], F32, tag="psown")
                nc.tensor.matmul(ps_own[:], ow_sb[:], gath[:],
                                 start=True, stop=True)
                musig = small.tile([S, 2 * C], F32, tag="musig")
                nc.vector.tensor_copy(musig[:], ps_own[:])
                ps_p = pmisc.tile([S, 2 * C], F32, tag="psp")
                nc.tensor.matmul(ps_p[:], pt_sb[:], gath[:],
                                 start=True, stop=True)
                prt = small.tile([S, 2 * C], F32, tag="prt")
                nc.vector.tensor_copy(prt[:], ps_p[:])

                nc.sync.dma_start(st_d[:], musig[:])

                # mix = a*(own - partner) + partner   for mu and sig jointly
                mix = small.tile([S, 2 * C], F32, tag="mix")
                nc.vector.tensor_tensor(out=mix[:], in0=musig[:], in1=prt[:],
                                        op=mybir.AluOpType.subtract)
                nc.vector.tensor_scalar(out=mix[:], in0=mix[:],
                                        scalar1=al_sb[:], scalar2=None,
                                        op0=mybir.AluOpType.mult)
                nc.vector.tensor_tensor(out=mix[:], in0=mix[:], in1=prt[:],
                                        op=mybir.AluOpType.add)

                # scale = sig_mix / sig ; bias = mu_mix - mu*scale  (rows)
                sb = small.tile([S, 2 * C], F32, tag="sb")
                scale = sb[:, 0:C]
                bias = sb[:, C:2 * C]
                rsig = small.tile([S, C], F32, tag="rsig")
                nc.vector.reciprocal(rsig[:], musig[:, C:2 * C])
                nc.vector.tensor_tensor(out=scale, in0=mix[:, C:2 * C],
                                        in1=rsig[:], op=mybir.AluOpType.mult)
                nc.vector.tensor_tensor(out=bias, in0=musig[:, 0:C],
                                        in1=scale,
                                        op=mybir.AluOpType.mult)
                nc.vector.tensor_tensor(out=bias, in0=mix[:, 0:C], in1=bias,
                                        op=mybir.AluOpType.subtract)

                # rows -> per-partition coef columns: pc[a*64+c, k*T+t] =
                # sb[2t+a, k*64+c] via 4 tiny selector matmuls; halves land
                # in [64, 2T] psum tiles, the upper half reaches partitions
                # 64-127 through a DRAM bounce (cross-partition moves are
                # only possible via DRAM on this path)
                pc = small.tile([P, 2 * T], F32, tag="pc")
                pcl = ppc.tile([64, 2 * T], F32, tag="pcl")
                pch = ppc.tile([64, 2 * T], F32, tag="pch")
                for a, dstp in ((0, pcl), (1, pch)):
                    for k in range(2):
                        nc.tensor.matmul(
                            dstp[:, bass.ts(k, T)],
                            sb[:, bass.ts(k, C)],
                            ra_sb[:, bass.ts(a, T)],
                            start=True, stop=True)
                nc.vector.tensor_copy(pc[0:64, :], pcl[:])
                tmpb = small.tile([64, 2 * T], F32, tag="tmpb")
                nc.vector.tensor_copy(tmpb[:], pch[:])
                nc.sync.dma_start(pc_d[:], tmpb[:])
                nc.sync.dma_start(pc[64:P, :], pc_d[:])

                # ---------------- phase C: apply ----------------
                if var == "stats":
                    return
                NH = N // 2
                for t in range(T):
                    for h in range(2):
                        sl = slice(h * NH, (h + 1) * NH)
                        nc.vector.tensor_scalar(
                            out=res[t][:, sl], in0=res[t][:, sl],
                            scalar1=pc[:, t:t + 1],
                            scalar2=pc[:, T + t:T + t + 1],
                            op0=mybir.AluOpType.mult,
                            op1=mybir.AluOpType.add)
                        # half-tile stores alternate the two HWDGE rings
                        # so both engage from the first applied half
                        eng = nc.scalar if (2 * t + h) % 2 else nc.sync
                        eng.dma_start(out_d[t][:, sl], res[t][:, sl])

            for _ in range(reps):
                emit(True)
            if loop_iters:
                with tc.For_i(0, loop_iters, 1):
                    emit(False, variant)

    nc.compile()
    return nc


def host_partner_alpha(alpha_u, select_noise, domain_labels, class_labels):
    """Host-side partner selection (mirrors the reference exactly)."""
    alpha_u = np.asarray(alpha_u, dtype=np.float32).reshape(B)
    noise = np.asarray(select_noise, dtype=np.float32)
    dom = np.asarray(domain_labels).reshape(B)
    cls = np.asarray(class_labels).reshape(B)
    valid = (cls[:, None] == cls[None, :]) & (dom[:, None] != dom[None, :])
    scores = np.where(valid, noise, -np.inf)
    has_valid = valid.any(axis=1)
    idx = np.where(has_valid, np.argmax(scores, axis=1), np.arange(B))
    a = alpha_u * ALPHA_MAX
    return idx.astype(np.int64), a


def _ra_mat():
    # ra[s, a*T+t] = 1 iff s == 2t+a  (selector rhs for the pc matmuls)
    ra = np.zeros((S, 2 * T), dtype=np.float32)
    for t in range(T):
        for a in range(2):
            ra[2 * t + a, a * T + t] = 1.0
    return ra


def make_in_maps(x, alpha_u, select_noise, domain_labels, class_labels):
    x16 = np.asarray(x).astype(np.float16)
    N = x16.shape[1]
    idx, a = host_partner_alpha(alpha_u, select_noise, domain_labels,
                                class_labels)
    ra = _ra_mat()
    in_maps = []
    for r in range(R):
        lo = r * S
        pt = np.zeros((B, S), dtype=np.float32)
        pt[idx[lo:lo + S], np.arange(S)] = 1.0
        ow = np.zeros((B, S), dtype=np.float32)
        ow[lo + np.arange(S), np.arange(S)] = 1.0
        # paired layout: [T, 128, N], partition a*64+c = ch c of sample 2t+a
        xt = np.ascontiguousarray(
            x16[lo:lo + S].transpose(0, 2, 1)).reshape(T, P, N)
        in_maps.append({
            "xs": xt,
            "pt": pt,
            "ow": ow,
            "ra": ra,
            "ey": np.eye(P, dtype=np.float32),
            "al": a[lo:lo + S].reshape(S, 1).astype(np.float32),
        })
    return in_maps


_NC_CACHE = {}


def _get_nc(N=16384):
    key = N
    if key not in _NC_CACHE:
        _NC_CACHE[key] = build_nc(N=N)
    return _NC_CACHE[key]


def kernel(x, alpha_u, select_noise, domain_labels, class_labels):
    x = np.asarray(x)
    Bx, N, Cx = x.shape
    assert Bx == B and Cx == C
    in_maps = make_in_maps(x, alpha_u, select_noise, domain_labels,
                           class_labels)

    nc = _get_nc(N=N)
    res = run_bass_kernel_spmd(nc, in_maps, core_ids=list(range(R)))
    global LAST_RESULTS
    LAST_RESULTS = res
    # un-pair + un-transpose: [T, 128, N] -> [S, N, C]
    outs = []
    for r in range(R):
        ot = np.asarray(res.results[r]["out"], dtype=np.float32)
        outs.append(ot.reshape(S, C, N).transpose(0, 2, 1))
    return np.ascontiguousarray(np.concatenate(outs, axis=0))


LAST_RESULTS = None
